# revision 22
# baseline (speedup 1.0000x reference)
"""Trainium2 Bass kernel for nn_HadaMard: fused proj + 2xLayerNorm + outer product.

Reference computation (per batch b, N = H*W = 1024):
  qf = q[b].reshape(C1, N)            # [1024, 1024]
  proj = Wp @ qf + bp                 # [256, 1024]
  qn = LN_d(proj) * g1 + b1           # LN over the 256-channel dim
  xn = LN_e(x[b]) * g2 + b2           # LN over the 32-channel dim
  out[d*32+e, n] = qn[d, n] * xn[e, n]    # [8192, 1024]

Sharding: data-parallel over B=8, one batch per NeuronCore.

Layout: TRANSPOSED on chip — partitions = spatial n (8 blocks of 128),
free dim = channels. Both LayerNorms become free-dim reductions and the
outer product becomes per-partition scalar multiplies:

  projT[n, d] = sum_c q[c, n] * WpT[c, d]      (PE, bf16, q natural = lhsT)
  q stats: bn_stats/bn_aggr (DVE); x stats precomputed during load fill
  sd = sqrt((var_q+eps)*(var_x+eps))           (ACT Sqrt, [128,1])
  qnT = projT - m_q (unnormalized bf16, ACT);  xn = (xT - m_x)/sd (Pool
  normalize_recip) so both LN scales live in the x-side per-e scalars:
  out[n, e*256+d] = qnT[n, d] * xn[n, e]       # 32 tensor_scalar ops per
                                               # block, split DVE(4x)/Pool
Host reassembles [N, e*256+d] bf16 -> [Cp*C2, H, W] f32.

DMA transfer time serializes on the issuing engine's queue (HWDGE on
SP/ACT, SWDGE on Pool; hw allows only those three), so loads and the 4
per-block store chunks are spread across SP/ACT/Pool to balance busy time,
and the matmul k-order follows chunk-arrival order to start PE early.
"""

import numpy as np

_CACHE = {}

B, C1, H, W = 8, 1024, 32, 32
C2 = 32
Cp = 256
N = H * W  # 1024
CD = Cp * C2  # 8192
NBLK = N // 128  # 8
KCH = C1 // 128  # 8
EPS = 1e-5


def _build_nc(simple):
    import os

    import concourse.bacc as bacc
    import concourse.bass as bass
    import concourse.mybir as mybir
    import concourse.tile as tile

    F32 = mybir.dt.float32
    F32R = mybir.dt.float32r
    BF16 = mybir.dt.bfloat16
    MULT = mybir.AluOpType.mult
    ADD = mybir.AluOpType.add
    SUB = mybir.AluOpType.subtract
    COPY = mybir.ActivationFunctionType.Copy
    IDENT = mybir.ActivationFunctionType.Identity
    SQRT = mybir.ActivationFunctionType.Sqrt

    def wrr(counts_str, keys, n):
        """Weighted round-robin list of n engine chars from comma counts."""
        cnt = dict(zip(keys, (int(v) for v in counts_str.split(","))))
        out, used = [], {k: 0 for k in keys}
        for _ in range(n):
            c = min((k for k in keys if cnt[k]),
                    key=lambda k: (used[k] + 1) / cnt[k])
            used[c] += 1
            out.append(c)
        return out

    # engine split of the 32 per-e output multiplies (DVE / Pool / ACT)
    nd = int(os.environ.get("HM_ND", "22"))
    npo = int(os.environ.get("HM_NP", "10"))
    na = 32 - nd - npo
    eng_of_e = wrr(f"{nd},{npo},{na}", "dpa", 32)
    # DMA engine maps (s=sync/SP, t=tensor/PE, d=DVE, p=Pool, a=ACT/scalar)
    qload = os.environ.get("HM_QLOAD", "apssssap")
    wload = os.environ.get("HM_WLOAD", "ap")
    xtload = os.environ.get("HM_XTLOAD", "s")
    # 32 store chunks (4 per block) from per-engine counts s,t,a,p,d;
    # 't' chunks are deferred so they don't block later matmuls in PE's stream
    stores = wrr(os.environ.get("HM_STORE_COUNTS", "15,0,12,5,0"), "stapd", 32)
    stores[28:32] = list(os.environ.get("HM_LAST_STORES", "saps"))
    # matmul accumulation order follows chunk-arrival order of the q loads
    korder = [int(c) for c in os.environ.get("HM_KORDER", "20134675")]
    looka = int(os.environ.get("HM_LOOKAHEAD", "2"))
    xdge = os.environ.get("HM_XDGE", "0") == "1"

    qbf16 = os.environ.get("HM_QBF16", "1") == "1"
    QDT = BF16 if qbf16 else F32

    nc = bacc.Bacc(None, target_bir_lowering=False)
    if xdge:
        for eng in (mybir.EngineType.DVE, mybir.EngineType.PE):
            if eng not in nc.hwdge_engines:
                nc.hwdge_engines.add(eng)
                nc.m.queues.append(
                    mybir.DMAQueue(
                        type="dynamic",
                        name=f"q{bass.shorten_engine_name(eng.name)}DynamicHW",
                        blocks=[],
                        engine=eng,
                        location_alt=False,
                        num_queues=16,
                        is_HWDGE=True,
                        num_semaphores=0,
                        semaphores=[],
                    )
                )

    def dma_eng(c):
        return {"s": nc.sync, "t": nc.tensor, "d": nc.vector,
                "p": nc.gpsimd, "a": nc.scalar}[c]

    q_d = nc.dram_tensor("q", [C1, N], QDT, kind="ExternalInput")
    w_d = nc.dram_tensor("w", [128, KCH * Cp], QDT, kind="ExternalInput")
    xt_d = nc.dram_tensor("xt", [128, NBLK * C2], F32, kind="ExternalInput")
    if not simple:
        bp_d = nc.dram_tensor("bpr", [128, Cp], F32, kind="ExternalInput")
        g1_d = nc.dram_tensor("g1r", [128, Cp], BF16, kind="ExternalInput")
        b1_d = nc.dram_tensor("b1r", [128, Cp], BF16, kind="ExternalInput")
        g2_d = nc.dram_tensor("g2r", [128, C2], F32, kind="ExternalInput")
        b2_d = nc.dram_tensor("b2r", [128, C2], F32, kind="ExternalInput")
    out_d = nc.dram_tensor("out", [N, CD], BF16, kind="ExternalOutput")

    with tile.TileContext(nc) as tc:
        with (
            tc.tile_pool(name="cst", bufs=1) as cst,
            tc.tile_pool(name="stt", bufs=4) as stt,
            tc.tile_pool(name="sml", bufs=16) as sml,
            tc.tile_pool(name="ost", bufs=5) as ost,
            tc.tile_pool(name="ps", bufs=8, space=bass.MemorySpace.PSUM) as ps,
        ):
            # ---- input loads (spread across engine DMA queues) ----
            xt_sb = cst.tile([128, NBLK * C2], F32, tag="xt")
            dma_eng(xtload[0]).dma_start(xt_sb[:], xt_d[:])
            # w first on its engines: every matmul needs it
            w_sb = cst.tile([128, KCH * Cp], QDT, tag="w")
            half = KCH * Cp // 2
            dma_eng(wload[0]).dma_start(w_sb[:, :half], w_d[:, :half])
            dma_eng(wload[1]).dma_start(w_sb[:, half:], w_d[:, half:])
            q_sb = []
            for k in range(KCH):
                _qt = cst.tile([128, N], QDT, tag=f"q{k}")
                q_sb.append(_qt)
                dma_eng(qload[k]).dma_start(
                    _qt[:], q_d[128 * k : 128 * (k + 1), :]
                )
            if not simple:
                bp_sb = cst.tile([128, Cp], F32, tag="bp")
                nc.sync.dma_start(bp_sb[:], bp_d[:])
                g1_sb = cst.tile([128, Cp], BF16, tag="g1")
                nc.sync.dma_start(g1_sb[:], g1_d[:])
                b1_sb = cst.tile([128, Cp], BF16, tag="b1")
                nc.sync.dma_start(b1_sb[:], b1_d[:])
                g2_sb = cst.tile([128, C2], F32, tag="g2")
                nc.sync.dma_start(g2_sb[:], g2_d[:])
                b2_sb = cst.tile([128, C2], F32, tag="b2")
                nc.sync.dma_start(b2_sb[:], b2_d[:])

            # prime the ACT function table (Sqrt/Identity/Copy share one set)
            prime = sml.tile([128, 1], F32, tag="prime")
            nc.vector.memset(prime[:], 1.0)
            nc.scalar.activation(prime[:], prime[:], SQRT)

            # Software-pipelined emission (engines execute their own streams
            # in order, so later-block prep is emitted ahead of earlier-block
            # bulk work):
            #   iter b: matmuls(b+1) | flush PE store chunks | stats(b+1) |
            #           e-ops(b) + inline store chunks(b)
            st = {}  # per-block state

            def emit_matmuls(blk):
                ns = slice(128 * blk, 128 * (blk + 1))
                pj = ps.tile([128, Cp], F32, tag="pj")
                for i, k in enumerate(korder):
                    lh = q_sb[k][:, ns]
                    rh = w_sb[:, Cp * k : Cp * (k + 1)]
                    if not qbf16:
                        lh, rh = lh.bitcast(F32R), rh.bitcast(F32R)
                    nc.tensor.matmul(
                        pj[:], lh, rh, start=(i == 0), stop=(i == KCH - 1)
                    )
                st[blk] = {"pj": pj}

            # x-side stats for ALL blocks depend only on xt: run during fill
            xside = []
            for blk in range(NBLK):
                xs = xt_sb[:, C2 * blk : C2 * (blk + 1)]
                st6x = sml.tile([128, 6], F32, tag=f"st6x{blk}")
                nc.vector.bn_stats(st6x[:], xs)
                mvx = sml.tile([128, 2], F32, tag=f"mvx{blk}")
                nc.vector.bn_aggr(mvx[:], st6x[:])
                vxe = sml.tile([128, 1], F32, tag=f"vxe{blk}")
                nc.gpsimd.tensor_scalar_add(vxe[:], mvx[:, 1:2], EPS)
                xfold = sml.tile([128, C2], F32, tag=f"xf{blk}")
                nc.gpsimd.tensor_scalar_sub(xfold[:], xs, mvx[:, 0:1])
                xside.append((vxe, xfold))

            def emit_stats(blk):
                s = st[blk]
                if simple:
                    pjv = s["pj"]
                else:
                    pjv = stt.tile([128, Cp], F32, tag="pjs")
                    nc.vector.tensor_add(pjv[:], s["pj"][:], bp_sb[:])
                s["pjv"] = pjv
                # q-side stats (DVE)
                st6 = sml.tile([128, 6], F32, tag="st6")
                nc.vector.bn_stats(st6[:], pjv[:])
                mv = sml.tile([128, 2], F32, tag="mv")
                nc.vector.bn_aggr(mv[:], st6[:])
                vxe, xfold = xside[blk]
                negmq = sml.tile([128, 1], F32, tag="ngm")
                nc.gpsimd.tensor_scalar_mul(negmq[:], mv[:, 0:1], -1.0)
                if simple:
                    vqe = sml.tile([128, 1], F32, tag="vqe")
                    nc.gpsimd.tensor_scalar_add(vqe[:], mv[:, 1:2], EPS)
                    vprod = sml.tile([128, 1], F32, tag="vp")
                    nc.gpsimd.tensor_tensor(vprod[:], vqe[:], vxe[:], op=MULT)
                    # qnT = projT - m_q (unnormalized; both LN scales live in
                    # the x-side per-e scalars) -> bf16
                    qnT = stt.tile([128, Cp], BF16, tag="qn")
                    nc.scalar.activation(qnT[:], pjv[:], IDENT, bias=negmq[:])
                    sd = sml.tile([128, 1], F32, tag="sd")
                    nc.scalar.activation(sd[:], vprod[:], SQRT)
                    # xn2 = (xs - m_x)/sd; sd becomes 1/sd in place (unused)
                    xn = sml.tile([128, C2], F32, tag="xn")
                    nc.gpsimd.normalize_recip(xn[:], xfold[:], sd[:])
                else:
                    vqeq = sml.tile([128, 1], F32, tag="vqeq")
                    nc.gpsimd.tensor_scalar_add(vqeq[:], mv[:, 1:2], EPS)
                    sdq = sml.tile([128, 1], F32, tag="sdq")
                    nc.scalar.activation(sdq[:], vqeq[:], SQRT)
                    rsq = sml.tile([128, 1], F32, tag="rsq")
                    nc.vector.reciprocal(rsq[:], sdq[:])
                    negm = sml.tile([128, 1], F32, tag="ngm2")
                    nc.gpsimd.tensor_tensor(negm[:], negmq[:], rsq[:], op=MULT)
                    qn0 = stt.tile([128, Cp], BF16, tag="qn0")
                    nc.scalar.activation(
                        qn0[:], pjv[:], IDENT, bias=negm[:], scale=rsq[:]
                    )
                    qnT = stt.tile([128, Cp], BF16, tag="qn")
                    nc.vector.tensor_tensor(qnT[:], qn0[:], g1_sb[:], op=MULT)
                    nc.vector.tensor_tensor(qnT[:], qnT[:], b1_sb[:], op=ADD)
                    sdx = sml.tile([128, 1], F32, tag="sdx")
                    nc.scalar.activation(sdx[:], vxe[:], SQRT)
                    xn = sml.tile([128, C2], F32, tag="xn")
                    nc.gpsimd.normalize_recip(xn[:], xfold[:], sdx[:])
                    nc.gpsimd.tensor_tensor(xn[:], xn[:], g2_sb[:], op=MULT)
                    nc.gpsimd.tensor_tensor(xn[:], xn[:], b2_sb[:], op=ADD)
                s["qnT"] = qnT
                s["xn"] = xn

            pe_chunks = []

            def flush_pe_chunks():
                for blk, g in pe_chunks:
                    cs = slice(2048 * g, 2048 * (g + 1))
                    nc.tensor.dma_start(
                        out_d[128 * blk : 128 * (blk + 1), cs],
                        st[blk]["ob"][:, cs],
                    )
                pe_chunks.clear()

            def emit_eops(blk):
                s = st[blk]
                ns = slice(128 * blk, 128 * (blk + 1))
                ob = ost.tile([128, CD], BF16, tag="ob")
                s["ob"] = ob
                qnT, xn = s["qnT"], s["xn"]
                for g in range(4):
                    for e in range(8 * g, 8 * (g + 1)):
                        osl = ob[:, Cp * e : Cp * (e + 1)]
                        sc = xn[:, e : e + 1]
                        c = eng_of_e[e]
                        if c == "d":
                            nc.vector.tensor_scalar_mul(osl, qnT[:], sc)
                        elif c == "p":
                            nc.gpsimd.tensor_scalar_mul(osl, qnT[:], sc)
                        else:
                            nc.scalar.activation(osl, qnT[:], COPY, scale=sc)
                    ch = stores[4 * blk + g]
                    cs = slice(2048 * g, 2048 * (g + 1))
                    if ch == "t":
                        pe_chunks.append((blk, g))
                    else:
                        dma_eng(ch).dma_start(out_d[ns, cs], ob[:, cs])

            for b in range(looka):
                emit_matmuls(b)
                emit_stats(b)
            for blk in range(NBLK):
                if blk + looka < NBLK:
                    emit_matmuls(blk + looka)
                flush_pe_chunks()
                if blk + looka < NBLK:
                    emit_stats(blk + looka)
                emit_eops(blk)
            flush_pe_chunks()

    nc.compile()
    return nc


def _host_inputs(q, x, Wp, bp, g1, b1, g2, b2):
    """Build the 8 per-core input maps."""
    import os

    import ml_dtypes

    simple = os.environ.get("HM_SIMPLE", "0") == "1"
    qbf16 = os.environ.get("HM_QBF16", "1") == "1"
    qdt = ml_dtypes.bfloat16 if qbf16 else np.float32
    qf = np.ascontiguousarray(np.asarray(q, dtype=np.float32).reshape(B, C1, N).astype(qdt))
    xf = np.asarray(x, dtype=np.float32).reshape(B, C2, N)
    # xt[p, blk*32+e] = x[e, blk*128+p]
    xt = np.ascontiguousarray(
        xf.reshape(B, C2, NBLK, 128).transpose(0, 3, 2, 1).reshape(B, 128, NBLK * C2)
    )
    # w[p, k*256+d] = WpT[k*128+p, d] = Wp[d, k*128+p]
    wpt = np.asarray(Wp, dtype=np.float32).T.reshape(KCH, 128, Cp)
    wpk = np.ascontiguousarray(wpt.transpose(1, 0, 2).reshape(128, KCH * Cp).astype(qdt))
    in_maps = []
    for b in range(B):
        m = {"q": qf[b], "w": wpk, "xt": xt[b]}
        if not simple:
            ones = np.ones((128, 1), dtype=np.float32)
            m["bpr"] = np.ascontiguousarray(ones * np.asarray(bp, np.float32)[None, :])
            m["g1r"] = np.ascontiguousarray(
                (ones * np.asarray(g1, np.float32)[None, :]).astype(ml_dtypes.bfloat16)
            )
            m["b1r"] = np.ascontiguousarray(
                (ones * np.asarray(b1, np.float32)[None, :]).astype(ml_dtypes.bfloat16)
            )
            m["g2r"] = np.ascontiguousarray(ones * np.asarray(g2, np.float32)[None, :])
            m["b2r"] = np.ascontiguousarray(ones * np.asarray(b2, np.float32)[None, :])
        in_maps.append(m)
    return in_maps


def _run(in_maps, trace=False):
    import os

    from concourse.bass_utils import run_bass_kernel_spmd

    simple = os.environ.get("HM_SIMPLE", "0") == "1"
    key = "nc" + ("1" if simple else "0")
    if key not in _CACHE:
        _CACHE[key] = _build_nc(simple)
    nc = _CACHE[key]
    res = run_bass_kernel_spmd(nc, in_maps, core_ids=list(range(B)), trace=trace)
    return res


def kernel(q, x, Wp, bp, g1, b1, g2, b2):
    import os

    simple = (
        np.allclose(np.asarray(bp), 0)
        and np.allclose(np.asarray(g1), 1)
        and np.allclose(np.asarray(b1), 0)
        and np.allclose(np.asarray(g2), 1)
        and np.allclose(np.asarray(b2), 0)
    )
    os.environ["HM_SIMPLE"] = "1" if simple else "0"
    in_maps = _host_inputs(q, x, Wp, bp, g1, b1, g2, b2)
    res = _run(in_maps, trace=False)
    # out[n, e*256+d] -> [d*32+e, n] = [CD, H, W]
    out = np.stack(
        [
            np.asarray(res.results[b]["out"], dtype=np.float32)
            .reshape(N, C2, Cp)
            .transpose(2, 1, 0)
            .reshape(CD, H, W)
            for b in range(B)
        ]
    ).astype(np.float32)
    _CACHE["last_res"] = res
    return out


# revision 23
# speedup vs baseline: 1.0025x; 1.0025x over previous
"""Trainium2 Bass kernel for nn_HadaMard: fused proj + 2xLayerNorm + outer product.

Reference computation (per batch b, N = H*W = 1024):
  qf = q[b].reshape(C1, N)            # [1024, 1024]
  proj = Wp @ qf + bp                 # [256, 1024]
  qn = LN_d(proj) * g1 + b1           # LN over the 256-channel dim
  xn = LN_e(x[b]) * g2 + b2           # LN over the 32-channel dim
  out[d*32+e, n] = qn[d, n] * xn[e, n]    # [8192, 1024]

Sharding: data-parallel over B=8, one batch per NeuronCore.

Layout: TRANSPOSED on chip — partitions = spatial n (8 blocks of 128),
free dim = channels. Both LayerNorms become free-dim reductions and the
outer product becomes per-partition scalar multiplies:

  projT[n, d] = sum_c q[c, n] * WpT[c, d]      (PE, bf16, q natural = lhsT)
  q stats: bn_stats/bn_aggr (DVE); x stats precomputed during load fill
  sd = sqrt((var_q+eps)*(var_x+eps))           (ACT Sqrt, [128,1])
  qnT = projT - m_q (unnormalized bf16, ACT);  xn = (xT - m_x)/sd (Pool
  normalize_recip) so both LN scales live in the x-side per-e scalars:
  out[n, e*256+d] = qnT[n, d] * xn[n, e]       # 32 tensor_scalar ops per
                                               # block, split DVE(4x)/Pool
Host reassembles [N, e*256+d] bf16 -> [Cp*C2, H, W] f32.

DMA transfer time serializes on the issuing engine's queue (HWDGE on
SP/ACT, SWDGE on Pool; hw allows only those three), so loads and the 4
per-block store chunks are spread across SP/ACT/Pool to balance busy time,
and the matmul k-order follows chunk-arrival order to start PE early.
"""

import numpy as np

_CACHE = {}

B, C1, H, W = 8, 1024, 32, 32
C2 = 32
Cp = 256
N = H * W  # 1024
CD = Cp * C2  # 8192
NBLK = N // 128  # 8
KCH = C1 // 128  # 8
EPS = 1e-5


def _build_nc(simple):
    import os

    import concourse.bacc as bacc
    import concourse.bass as bass
    import concourse.mybir as mybir
    import concourse.tile as tile

    F32 = mybir.dt.float32
    F32R = mybir.dt.float32r
    BF16 = mybir.dt.bfloat16
    MULT = mybir.AluOpType.mult
    ADD = mybir.AluOpType.add
    SUB = mybir.AluOpType.subtract
    COPY = mybir.ActivationFunctionType.Copy
    IDENT = mybir.ActivationFunctionType.Identity
    SQRT = mybir.ActivationFunctionType.Sqrt

    def wrr(counts_str, keys, n):
        """Weighted round-robin list of n engine chars from comma counts."""
        cnt = dict(zip(keys, (int(v) for v in counts_str.split(","))))
        out, used = [], {k: 0 for k in keys}
        for _ in range(n):
            c = min((k for k in keys if cnt[k]),
                    key=lambda k: (used[k] + 1) / cnt[k])
            used[c] += 1
            out.append(c)
        return out

    # engine split of the 32 per-e output multiplies (DVE / Pool / ACT)
    nd = int(os.environ.get("HM_ND", "22"))
    npo = int(os.environ.get("HM_NP", "10"))
    na = 32 - nd - npo
    eng_of_e = wrr(f"{nd},{npo},{na}", "dpa", 32)
    # DMA engine maps (s=sync/SP, t=tensor/PE, d=DVE, p=Pool, a=ACT/scalar)
    qload = os.environ.get("HM_QLOAD", "apssssap")
    wload = os.environ.get("HM_WLOAD", "ap")
    xtload = os.environ.get("HM_XTLOAD", "s")
    # 32 store chunks (4 per block) from per-engine counts s,t,a,p,d;
    # 't' chunks are deferred so they don't block later matmuls in PE's stream
    stores = wrr(os.environ.get("HM_STORE_COUNTS", "15,0,13,4,0"), "stapd", 32)
    stores[28:32] = list(os.environ.get("HM_LAST_STORES", "saps"))
    # matmul accumulation order follows chunk-arrival order of the q loads
    korder = [int(c) for c in os.environ.get("HM_KORDER", "20134675")]
    looka = int(os.environ.get("HM_LOOKAHEAD", "2"))
    xdge = os.environ.get("HM_XDGE", "0") == "1"

    qbf16 = os.environ.get("HM_QBF16", "1") == "1"
    QDT = BF16 if qbf16 else F32

    nc = bacc.Bacc(None, target_bir_lowering=False)
    if xdge:
        for eng in (mybir.EngineType.DVE, mybir.EngineType.PE):
            if eng not in nc.hwdge_engines:
                nc.hwdge_engines.add(eng)
                nc.m.queues.append(
                    mybir.DMAQueue(
                        type="dynamic",
                        name=f"q{bass.shorten_engine_name(eng.name)}DynamicHW",
                        blocks=[],
                        engine=eng,
                        location_alt=False,
                        num_queues=16,
                        is_HWDGE=True,
                        num_semaphores=0,
                        semaphores=[],
                    )
                )

    def dma_eng(c):
        return {"s": nc.sync, "t": nc.tensor, "d": nc.vector,
                "p": nc.gpsimd, "a": nc.scalar}[c]

    q_d = nc.dram_tensor("q", [C1, N], QDT, kind="ExternalInput")
    w_d = nc.dram_tensor("w", [128, KCH * Cp], QDT, kind="ExternalInput")
    xt_d = nc.dram_tensor("xt", [128, NBLK * C2], F32, kind="ExternalInput")
    if not simple:
        bp_d = nc.dram_tensor("bpr", [128, Cp], F32, kind="ExternalInput")
        g1_d = nc.dram_tensor("g1r", [128, Cp], BF16, kind="ExternalInput")
        b1_d = nc.dram_tensor("b1r", [128, Cp], BF16, kind="ExternalInput")
        g2_d = nc.dram_tensor("g2r", [128, C2], F32, kind="ExternalInput")
        b2_d = nc.dram_tensor("b2r", [128, C2], F32, kind="ExternalInput")
    out_d = nc.dram_tensor("out", [N, CD], BF16, kind="ExternalOutput")

    with tile.TileContext(nc) as tc:
        with (
            tc.tile_pool(name="cst", bufs=1) as cst,
            tc.tile_pool(name="stt", bufs=4) as stt,
            tc.tile_pool(name="sml", bufs=16) as sml,
            tc.tile_pool(name="ost", bufs=5) as ost,
            tc.tile_pool(name="ps", bufs=8, space=bass.MemorySpace.PSUM) as ps,
        ):
            # ---- input loads (spread across engine DMA queues) ----
            xt_sb = cst.tile([128, NBLK * C2], F32, tag="xt")
            dma_eng(xtload[0]).dma_start(xt_sb[:], xt_d[:])
            # w first on its engines: every matmul needs it
            w_sb = cst.tile([128, KCH * Cp], QDT, tag="w")
            half = KCH * Cp // 2
            dma_eng(wload[0]).dma_start(w_sb[:, :half], w_d[:, :half])
            dma_eng(wload[1]).dma_start(w_sb[:, half:], w_d[:, half:])
            q_sb = []
            for k in range(KCH):
                _qt = cst.tile([128, N], QDT, tag=f"q{k}")
                q_sb.append(_qt)
                dma_eng(qload[k]).dma_start(
                    _qt[:], q_d[128 * k : 128 * (k + 1), :]
                )
            if not simple:
                bp_sb = cst.tile([128, Cp], F32, tag="bp")
                nc.sync.dma_start(bp_sb[:], bp_d[:])
                g1_sb = cst.tile([128, Cp], BF16, tag="g1")
                nc.sync.dma_start(g1_sb[:], g1_d[:])
                b1_sb = cst.tile([128, Cp], BF16, tag="b1")
                nc.sync.dma_start(b1_sb[:], b1_d[:])
                g2_sb = cst.tile([128, C2], F32, tag="g2")
                nc.sync.dma_start(g2_sb[:], g2_d[:])
                b2_sb = cst.tile([128, C2], F32, tag="b2")
                nc.sync.dma_start(b2_sb[:], b2_d[:])

            # prime the ACT function table (Sqrt/Identity/Copy share one set)
            prime = sml.tile([128, 1], F32, tag="prime")
            nc.vector.memset(prime[:], 1.0)
            nc.scalar.activation(prime[:], prime[:], SQRT)

            # Software-pipelined emission (engines execute their own streams
            # in order, so later-block prep is emitted ahead of earlier-block
            # bulk work):
            #   iter b: matmuls(b+1) | flush PE store chunks | stats(b+1) |
            #           e-ops(b) + inline store chunks(b)
            st = {}  # per-block state

            def emit_matmuls(blk):
                ns = slice(128 * blk, 128 * (blk + 1))
                pj = ps.tile([128, Cp], F32, tag="pj")
                for i, k in enumerate(korder):
                    lh = q_sb[k][:, ns]
                    rh = w_sb[:, Cp * k : Cp * (k + 1)]
                    if not qbf16:
                        lh, rh = lh.bitcast(F32R), rh.bitcast(F32R)
                    nc.tensor.matmul(
                        pj[:], lh, rh, start=(i == 0), stop=(i == KCH - 1)
                    )
                st[blk] = {"pj": pj}

            # x-side stats for ALL blocks depend only on xt: run during fill
            xside = []
            for blk in range(NBLK):
                xs = xt_sb[:, C2 * blk : C2 * (blk + 1)]
                st6x = sml.tile([128, 6], F32, tag=f"st6x{blk}")
                nc.vector.bn_stats(st6x[:], xs)
                mvx = sml.tile([128, 2], F32, tag=f"mvx{blk}")
                nc.vector.bn_aggr(mvx[:], st6x[:])
                vxe = sml.tile([128, 1], F32, tag=f"vxe{blk}")
                nc.gpsimd.tensor_scalar_add(vxe[:], mvx[:, 1:2], EPS)
                xfold = sml.tile([128, C2], F32, tag=f"xf{blk}")
                nc.gpsimd.tensor_scalar_sub(xfold[:], xs, mvx[:, 0:1])
                xside.append((vxe, xfold))

            def emit_stats(blk):
                s = st[blk]
                if simple:
                    pjv = s["pj"]
                else:
                    pjv = stt.tile([128, Cp], F32, tag="pjs")
                    nc.vector.tensor_add(pjv[:], s["pj"][:], bp_sb[:])
                s["pjv"] = pjv
                # q-side stats (DVE)
                st6 = sml.tile([128, 6], F32, tag="st6")
                nc.vector.bn_stats(st6[:], pjv[:])
                mv = sml.tile([128, 2], F32, tag="mv")
                nc.vector.bn_aggr(mv[:], st6[:])
                vxe, xfold = xside[blk]
                negmq = sml.tile([128, 1], F32, tag="ngm")
                nc.gpsimd.tensor_scalar_mul(negmq[:], mv[:, 0:1], -1.0)
                if simple:
                    vqe = sml.tile([128, 1], F32, tag="vqe")
                    nc.gpsimd.tensor_scalar_add(vqe[:], mv[:, 1:2], EPS)
                    vprod = sml.tile([128, 1], F32, tag="vp")
                    nc.gpsimd.tensor_tensor(vprod[:], vqe[:], vxe[:], op=MULT)
                    # qnT = projT - m_q (unnormalized; both LN scales live in
                    # the x-side per-e scalars) -> bf16
                    qnT = stt.tile([128, Cp], BF16, tag="qn")
                    nc.scalar.activation(qnT[:], pjv[:], IDENT, bias=negmq[:])
                    sd = sml.tile([128, 1], F32, tag="sd")
                    nc.scalar.activation(sd[:], vprod[:], SQRT)
                    # xn2 = (xs - m_x)/sd; sd becomes 1/sd in place (unused)
                    xn = sml.tile([128, C2], F32, tag="xn")
                    nc.gpsimd.normalize_recip(xn[:], xfold[:], sd[:])
                else:
                    vqeq = sml.tile([128, 1], F32, tag="vqeq")
                    nc.gpsimd.tensor_scalar_add(vqeq[:], mv[:, 1:2], EPS)
                    sdq = sml.tile([128, 1], F32, tag="sdq")
                    nc.scalar.activation(sdq[:], vqeq[:], SQRT)
                    rsq = sml.tile([128, 1], F32, tag="rsq")
                    nc.vector.reciprocal(rsq[:], sdq[:])
                    negm = sml.tile([128, 1], F32, tag="ngm2")
                    nc.gpsimd.tensor_tensor(negm[:], negmq[:], rsq[:], op=MULT)
                    qn0 = stt.tile([128, Cp], BF16, tag="qn0")
                    nc.scalar.activation(
                        qn0[:], pjv[:], IDENT, bias=negm[:], scale=rsq[:]
                    )
                    qnT = stt.tile([128, Cp], BF16, tag="qn")
                    nc.vector.tensor_tensor(qnT[:], qn0[:], g1_sb[:], op=MULT)
                    nc.vector.tensor_tensor(qnT[:], qnT[:], b1_sb[:], op=ADD)
                    sdx = sml.tile([128, 1], F32, tag="sdx")
                    nc.scalar.activation(sdx[:], vxe[:], SQRT)
                    xn = sml.tile([128, C2], F32, tag="xn")
                    nc.gpsimd.normalize_recip(xn[:], xfold[:], sdx[:])
                    nc.gpsimd.tensor_tensor(xn[:], xn[:], g2_sb[:], op=MULT)
                    nc.gpsimd.tensor_tensor(xn[:], xn[:], b2_sb[:], op=ADD)
                s["qnT"] = qnT
                s["xn"] = xn

            pe_chunks = []

            def flush_pe_chunks():
                for blk, g in pe_chunks:
                    cs = slice(2048 * g, 2048 * (g + 1))
                    nc.tensor.dma_start(
                        out_d[128 * blk : 128 * (blk + 1), cs],
                        st[blk]["ob"][:, cs],
                    )
                pe_chunks.clear()

            def emit_eops(blk):
                s = st[blk]
                ns = slice(128 * blk, 128 * (blk + 1))
                ob = ost.tile([128, CD], BF16, tag="ob")
                s["ob"] = ob
                qnT, xn = s["qnT"], s["xn"]
                for g in range(4):
                    for e in range(8 * g, 8 * (g + 1)):
                        osl = ob[:, Cp * e : Cp * (e + 1)]
                        sc = xn[:, e : e + 1]
                        c = eng_of_e[e]
                        if c == "d":
                            nc.vector.tensor_scalar_mul(osl, qnT[:], sc)
                        elif c == "p":
                            nc.gpsimd.tensor_scalar_mul(osl, qnT[:], sc)
                        else:
                            nc.scalar.activation(osl, qnT[:], COPY, scale=sc)
                    ch = stores[4 * blk + g]
                    cs = slice(2048 * g, 2048 * (g + 1))
                    if ch == "t":
                        pe_chunks.append((blk, g))
                    else:
                        dma_eng(ch).dma_start(out_d[ns, cs], ob[:, cs])

            for b in range(looka):
                emit_matmuls(b)
                emit_stats(b)
            for blk in range(NBLK):
                if blk + looka < NBLK:
                    emit_matmuls(blk + looka)
                flush_pe_chunks()
                if blk + looka < NBLK:
                    emit_stats(blk + looka)
                emit_eops(blk)
            flush_pe_chunks()

    nc.compile()
    return nc


def _host_inputs(q, x, Wp, bp, g1, b1, g2, b2):
    """Build the 8 per-core input maps."""
    import os

    import ml_dtypes

    simple = os.environ.get("HM_SIMPLE", "0") == "1"
    qbf16 = os.environ.get("HM_QBF16", "1") == "1"
    qdt = ml_dtypes.bfloat16 if qbf16 else np.float32
    qf = np.ascontiguousarray(np.asarray(q, dtype=np.float32).reshape(B, C1, N).astype(qdt))
    xf = np.asarray(x, dtype=np.float32).reshape(B, C2, N)
    # xt[p, blk*32+e] = x[e, blk*128+p]
    xt = np.ascontiguousarray(
        xf.reshape(B, C2, NBLK, 128).transpose(0, 3, 2, 1).reshape(B, 128, NBLK * C2)
    )
    # w[p, k*256+d] = WpT[k*128+p, d] = Wp[d, k*128+p]
    wpt = np.asarray(Wp, dtype=np.float32).T.reshape(KCH, 128, Cp)
    wpk = np.ascontiguousarray(wpt.transpose(1, 0, 2).reshape(128, KCH * Cp).astype(qdt))
    in_maps = []
    for b in range(B):
        m = {"q": qf[b], "w": wpk, "xt": xt[b]}
        if not simple:
            ones = np.ones((128, 1), dtype=np.float32)
            m["bpr"] = np.ascontiguousarray(ones * np.asarray(bp, np.float32)[None, :])
            m["g1r"] = np.ascontiguousarray(
                (ones * np.asarray(g1, np.float32)[None, :]).astype(ml_dtypes.bfloat16)
            )
            m["b1r"] = np.ascontiguousarray(
                (ones * np.asarray(b1, np.float32)[None, :]).astype(ml_dtypes.bfloat16)
            )
            m["g2r"] = np.ascontiguousarray(ones * np.asarray(g2, np.float32)[None, :])
            m["b2r"] = np.ascontiguousarray(ones * np.asarray(b2, np.float32)[None, :])
        in_maps.append(m)
    return in_maps


def _run(in_maps, trace=False):
    import os

    from concourse.bass_utils import run_bass_kernel_spmd

    simple = os.environ.get("HM_SIMPLE", "0") == "1"
    key = "nc" + ("1" if simple else "0")
    if key not in _CACHE:
        _CACHE[key] = _build_nc(simple)
    nc = _CACHE[key]
    res = run_bass_kernel_spmd(nc, in_maps, core_ids=list(range(B)), trace=trace)
    return res


def kernel(q, x, Wp, bp, g1, b1, g2, b2):
    import os

    simple = (
        np.allclose(np.asarray(bp), 0)
        and np.allclose(np.asarray(g1), 1)
        and np.allclose(np.asarray(b1), 0)
        and np.allclose(np.asarray(g2), 1)
        and np.allclose(np.asarray(b2), 0)
    )
    os.environ["HM_SIMPLE"] = "1" if simple else "0"
    in_maps = _host_inputs(q, x, Wp, bp, g1, b1, g2, b2)
    res = _run(in_maps, trace=False)
    # out[n, e*256+d] -> [d*32+e, n] = [CD, H, W]
    out = np.stack(
        [
            np.asarray(res.results[b]["out"], dtype=np.float32)
            .reshape(N, C2, Cp)
            .transpose(2, 1, 0)
            .reshape(CD, H, W)
            for b in range(B)
        ]
    ).astype(np.float32)
    _CACHE["last_res"] = res
    return out
